# revision 10
# baseline (speedup 1.0000x reference)
"""Causal multi-head self-attention on 8 Trainium2 NeuronCores.

Problem: x[4, 2048, 1024], 16 heads x d_k=64, torch-Linear-style projections
(weights stored [in, out]), causal softmax attention, output projection.

Sharding (SPMD, one program, per-core data):
  core c -> batch b = c // 2, head-group g = c % 2 (8 heads = 512 model cols).
  QKV column-parallel, output projection row-parallel; the 2-way partial sum
  of the output projection (+ b_o) is done on host at gather time.

Per-core device kernel. All matmul operands are fp16 (cast on host for the
DRAM-resident ones). Accumulation is always fp32 in PSUM.

v4 changes vs the 306us baseline:
  - ALL matmul inputs are host-packed into ONE partition-contiguous DRAM
    tensor, in consumption order, and loaded into one SBUF buffer by 7
    chained big DMAs (each overlaps the previous by one column, so the WAW
    dependency serializes them: every chunk moves at full ~430GB/s instead
    of fair-sharing with later chunks).  The baseline's 101 small DMAs
    (1KB packets) made the first matmul wait 21us; here it starts at ~12us.
  - ~44 warm-up matmuls on the tri tile while the first DMA lands, so the
    PE HAM clock-gate is already 8/8 when real work starts.
  - Output staged in fp16 (tolerance 2e-2; fp16 partials cost ~1e-3) and
    written with 6 big DMAs; the final two are 0.5MB to shorten the tail.
  - Attention refactored into chunk-lists; pair 3's second half runs as two
    single-chunk passes with the out-projection of finished s-tiles emitted
    between them, so out-proj compute/DMA overlaps the attention tail.
  - Softmax normalization: reciprocal on the [1,512] denominator row (copied
    to SBUF first - custom-DVE ops misread PSUM) before the partition
    broadcast; saves a [64,512] DVE reciprocal per head-chunk.
"""

import sys

sys.path.insert(0, "/opt/trn_rl_repo")

from contextlib import ExitStack

import numpy as np

import concourse.bass as bass  # noqa: F401
import concourse.mybir as mybir
import concourse.tile as tile
from concourse import bacc, bass_utils
from concourse.masks import make_upper_triangular

F32 = mybir.dt.float32
F16 = mybir.dt.float16

B, S, D, H, DK = 4, 2048, 1024, 16, 64
NCORE = 8
HPC = 8  # heads per core
DPC = HPC * DK  # model cols per core = 512
NK = D // 128  # k-tiles over the model dim = 8
NST = S // 128  # 128-row S tiles = 16
NSC = S // 512  # 512-wide S chunks = 4
SCALE = 1.0 / float(np.sqrt(DK))
N_WARM = 44  # PE warm-up matmuls during the first input DMA

# Packed-input layout: one [128, INCOLS] fp16 tensor, consumption order.
# Entries: (kind, key, width-cols).  x(0,*) and wv interleave in k-halves so
# the first v-projection matmuls can start after the first 2MB chunk.
_LAYOUT = (
    [("x", (0, k), 512) for k in range(4)]
    + [("wv", k, 512) for k in range(4)]
    + [("x", (0, k), 512) for k in range(4, 8)]
    + [("wv", k, 512) for k in range(4, 8)]
    + [("x", (1, k), 512) for k in range(NK)]
    + [("wq", k, 512) for k in range(NK)]
    + [("wk", k, 512) for k in range(NK)]
    + [("x", (2, k), 512) for k in range(NK)]
    + [("x", (3, k), 512) for k in range(NK)]
    + [("wo", kk, 1024) for kk in range(4)]
)
COL = {}
_off = 0
for _kind, _key, _w in _LAYOUT:
    COL[(_kind, _key)] = _off
    _off += _w
INCOLS = _off  # 32768
# chained-DMA chunk boundaries (cols): x0+wv | x1 | wq+wk | x2 | x3 | wo
CHUNKS = [0, 4096, 8192, 12288, 20480, 24576, 28672, INCOLS]


def emit(nc, tc, ctx):
    allin = nc.dram_tensor("allin", [128, INCOLS], F16, kind="ExternalInput").ap()
    bqkin = nc.dram_tensor("bqkin", [128, 8], F32, kind="ExternalInput").ap()
    bv = nc.dram_tensor("bv", [DPC], F32, kind="ExternalInput").ap()
    # Output: [p, s_tile, n, 512] fp16; host unpacks + upcasts.
    outd = nc.dram_tensor("out", [128, NST, 2, 512], F16, kind="ExternalOutput").ap()

    singles = ctx.enter_context(tc.tile_pool(name="singles", bufs=1))

    # ---- constants / persistent SBUF ----
    tri = singles.tile([128, 128], F16, tag="tri", name="tri")  # 1 where sk<=sq
    make_upper_triangular(nc, tri, val=1.0, diag=True)
    ones8 = singles.tile([128, HPC], F32, tag="ones8", name="ones8")
    nc.vector.memset(ones8, 1.0)
    bqk_sb = singles.tile([128, 8], F32, tag="bqk", name="bqk")
    nc.sync.dma_start(out=bqk_sb, in_=bqkin)
    bv_row = singles.tile([1, DPC], F32, tag="bv_row", name="bv_row")
    nc.sync.dma_start(out=bv_row, in_=bv.rearrange("(o f) -> o f", o=1))
    bv_bc = singles.tile([128, DPC], F32, tag="bv_bc", name="bv_bc")
    nc.gpsimd.partition_broadcast(bv_bc, bv_row)
    # v_store[s]: [128, HPC, 128] fp16 zero-padded pv stationaries: per head
    # slot cols 0-63 = v, col 64 = 1.0 (denominator), cols 65-127 = 0.
    v_store = []
    for s in range(NST):
        t = singles.tile([128, HPC, 128], F16, tag=f"v{s}", name=f"v{s}")
        nc.vector.tensor_copy(out=t[:, :, DK:DK + 1],
                              in_=ones8.rearrange("p (h o) -> p h o", o=1))
        v_store.append(t)
    # kT packed per head pair j: [128, S] (rows 0-63 head 2j, rest 2j+1).
    # qT unpacked per head with the OTHER head's 64 rows zeroed, so scores
    # run at K=128 (full array) against the packed kT.
    kT = [singles.tile([128, S], F16, tag=f"kT{j}", name=f"kT{j}") for j in range(4)]
    qT = []
    for h in range(HPC):
        t = singles.tile([128, S], F16, tag=f"qTz{h}", name=f"qTz{h}")
        nc.gpsimd.memset(t[(1 - h % 2) * DK:(2 - h % 2) * DK, :], 0.0)
        qT.append(t)
    # yT packed per head pair (filled in phase B, consumed in C)
    yT_pack = [singles.tile([128, S], F16, tag=f"yT{j}", name=f"yT{j}") for j in range(4)]

    # ---- interleaved projections + attention (per head pair) ----
    with tc.tile_pool(name="pa_sbuf", bufs=1) as pa, \
         tc.tile_pool(name="pa_ps", bufs=2, space="PSUM") as aps, \
         tc.tile_pool(name="pb_p", bufs=4) as p_pool, \
         tc.tile_pool(name="pb_div", bufs=2) as div_pool, \
         tc.tile_pool(name="pc_stage", bufs=2) as ostage_pool, \
         tc.tile_pool(name="pb_sps", bufs=2, space="PSUM") as sps, \
         tc.tile_pool(name="pb_yps", bufs=2, space="PSUM") as yps:
        # PE warm-up: back-to-back garbage matmuls on the tri tile into a
        # scratch PSUM bank so the HAM clock-gate reaches 8/8 during the
        # input DMA (first real matmul then runs at 2.4GHz, not 1.2).
        warm_ps = aps.tile([128, 512], F32, tag="a", name="aps")
        for _ in range(N_WARM):
            nc.tensor.matmul(warm_ps[:, 0:128], lhsT=tri, rhs=tri,
                             start=True, stop=True)

        # Chained input DMAs: chunk i+1's dest overlaps chunk i's last column
        # (same source data), so the WAW dep serializes the transfers and
        # each chunk gets the full DMA bandwidth, in consumption order.
        ibuf = pa.tile([128, INCOLS], F16, tag="ibuf", name="ibuf")
        for ci in range(len(CHUNKS) - 1):
            b0, b1 = CHUNKS[ci], CHUNKS[ci + 1]
            e = min(b1 + 1, INCOLS)
            nc.sync.dma_start(out=ibuf[:, b0:e], in_=allin[:, b0:e])

        def xat(c, k):
            o = COL[("x", (c, k))]
            return ibuf[:, o:o + 512]

        def emit_qk_proj(j, cs):
            for c in cs:
                pq = aps.tile([128, 512], F32, tag="a", name="aps")
                pk = aps.tile([128, 512], F32, tag="a", name="aps")
                for k in range(NK):
                    o = COL[("wq", k)] + j * 128
                    nc.tensor.matmul(pq, lhsT=ibuf[:, o:o + 128], rhs=xat(c, k),
                                     start=(k == 0), stop=(k == NK - 1))
                for k in range(NK):
                    o = COL[("wk", k)] + j * 128
                    nc.tensor.matmul(pk, lhsT=ibuf[:, o:o + 128], rhs=xat(c, k),
                                     start=(k == 0), stop=(k == NK - 1))
                for hh in range(2):
                    nc.vector.tensor_scalar_add(
                        out=qT[2 * j + hh][hh * DK:(hh + 1) * DK, c * 512:(c + 1) * 512],
                        in0=pq[hh * DK:(hh + 1) * DK, :],
                        scalar1=bqk_sb[hh * DK:(hh + 1) * DK, j:j + 1])
                nc.vector.tensor_scalar_add(out=kT[j][:, c * 512:(c + 1) * 512],
                                            in0=pk, scalar1=bqk_sb[:, 4 + j:5 + j])

        def emit_v_proj(c):
            for si in range(4):
                s = c * 4 + si
                ps = aps.tile([128, 512], F32, tag="a", name="aps")
                for k in range(NK):
                    ox = COL[("x", (c, k))] + si * 128
                    ow = COL[("wv", k)]
                    nc.tensor.matmul(ps, lhsT=ibuf[:, ox:ox + 128],
                                     rhs=ibuf[:, ow:ow + 512],
                                     start=(k == 0), stop=(k == NK - 1))
                nc.vector.tensor_add(
                    out=v_store[s][:, :, 0:DK],
                    in0=ps.rearrange("p (h e) -> p h e", e=DK),
                    in1=bv_bc.rearrange("p (h e) -> p h e", e=DK))

        # v projection / first qk projection, interleaved in DMA-chain order
        emit_v_proj(0)
        emit_v_proj(1)
        emit_qk_proj(0, (0, 1))
        emit_v_proj(2)
        emit_qk_proj(0, (2,))
        emit_v_proj(3)
        emit_qk_proj(0, (3,))

        def emit_out_proj(s_lo, s_hi, gsize):
            # partial = y_cat @ wo for s-tiles [s_lo, s_hi), staged fp16 in
            # groups of gsize s-tiles -> one DMA per group.
            for g0 in range(s_lo, s_hi, gsize):
                stg = ostage_pool.tile([128, 4, 2, 512], F16, tag="o", name="ostage")
                for si2 in range(gsize):
                    s = g0 + si2
                    for n in range(2):
                        ps = aps.tile([128, 512], F32, tag="a", name="aps")
                        for kk in range(4):
                            o = COL[("wo", kk)] + n * 512
                            nc.tensor.matmul(ps,
                                             lhsT=yT_pack[kk][:, s * 128:(s + 1) * 128],
                                             rhs=ibuf[:, o:o + 512],
                                             start=(kk == 0), stop=(kk == 3))
                        if (2 * si2 + n) % 2 == 0:
                            nc.vector.tensor_copy(out=stg[:, si2, n], in_=ps)
                        else:
                            nc.scalar.copy(out=stg[:, si2, n], in_=ps)
                nc.sync.dma_start(out=outd[:, g0:g0 + gsize], in_=stg[:, 0:gsize])

        def attn_block(j, chunks):
            # attention for the two heads of pair j over the given list of
            # consecutive sq 512-chunks.
            base = chunks[0] * 512
            wdt = 512 * len(chunks)
            for h in (2 * j, 2 * j + 1):
                po = (h % 2) * DK  # partition offset in packed q/k/y tiles
                y_ps = {c: yps.tile([128, 512], F32, tag="y", name="yps")
                        for c in chunks}
                for a in range(4 * chunks[-1] + 4):  # sk tiles
                    lo = max(0, a * 128 - base)  # valid span start in block
                    has_diag = a * 128 >= base
                    st = sps.tile([128, 1024], F32, tag="s", name="sps")
                    pt = p_pool.tile([128, 1024], F16, tag="p", name="pt")
                    for c in chunks:
                        x0 = max(lo, (c - chunks[0]) * 512)
                        x1 = (c - chunks[0] + 1) * 512
                        if x0 >= x1:
                            continue
                        nc.tensor.matmul(
                            st[:, x0:x1],
                            lhsT=kT[j][:, a * 128:(a + 1) * 128],
                            rhs=qT[h][:, base + x0:base + x1],
                            start=True, stop=True)
                    nc.scalar.activation(out=pt[:, lo:wdt], in_=st[:, lo:wdt],
                                         func=mybir.ActivationFunctionType.Exp,
                                         scale=SCALE)
                    if has_diag:  # zero p where sk_local > sq_local
                        nc.vector.tensor_mul(out=pt[:, lo:lo + 128],
                                             in0=pt[:, lo:lo + 128], in1=tri)
                    # [128, 65] = [v | 1]: 65-col stationary halves LDWEIGHTS
                    va = v_store[a][:, h, 0:DK + 1]
                    for c in chunks:
                        x0 = max(lo, (c - chunks[0]) * 512)
                        x1 = (c - chunks[0] + 1) * 512
                        if x0 >= x1:
                            continue
                        nc.tensor.matmul(
                            y_ps[c][0:DK + 1, x0 - (c - chunks[0]) * 512:512],
                            lhsT=va, rhs=pt[:, x0:x1],
                            start=(a == 0), stop=(a == 4 * c + 3))
                # scale yT rows by 1/denominator -> packed fp16 SBUF.  One
                # copy drains the PSUM bank first (frees it for the next
                # head's pv ~2us earlier); the reciprocal chain runs on SBUF.
                for c in chunks:
                    drow = div_pool.tile([1, 512], F32, tag="drow", name="drow")
                    nc.vector.tensor_copy(out=drow, in_=y_ps[c][DK:DK + 1, :])
                    ysb = div_pool.tile([DK, 512], F32, tag="ysb", name="ysb")
                    nc.vector.tensor_copy(out=ysb, in_=y_ps[c][0:DK, :])
                    rrow = div_pool.tile([1, 512], F32, tag="rrow", name="rrow")
                    # custom-DVE ops cannot partition-shift: drow is at
                    # partition 0, matching rrow (a base-64 input breaks on HW)
                    nc.vector.reciprocal_approx_fast(out=rrow, in_=drow)
                    rbc = div_pool.tile([DK, 512], F32, tag="rbc", name="rbc")
                    nc.gpsimd.partition_broadcast(rbc, rrow)
                    nc.vector.tensor_mul(out=yT_pack[j][po:po + DK, c * 512:(c + 1) * 512],
                                         in0=ysb[0:DK, :], in1=rbc)

        for j in range(4):
            if j > 0:
                emit_qk_proj(j, range(NSC))
            attn_block(j, [0, 1])
            if j < 3:
                attn_block(j, [2, 3])
        # pair 3 second half: two single-chunk passes with the finished
        # s-tiles' out-projection (compute + DMA) emitted between them.
        emit_out_proj(0, 8, 4)
        attn_block(3, [2])
        emit_out_proj(8, 12, 4)
        attn_block(3, [3])
        emit_out_proj(12, 16, 2)


_CACHED_NC = None


def build_program():
    global _CACHED_NC
    if _CACHED_NC is not None:
        return _CACHED_NC
    nc = bacc.Bacc("TRN2", target_bir_lowering=False, debug=False,
                   enable_asserts=False, num_devices=NCORE)
    with tile.TileContext(nc) as tc:
        with ExitStack() as ctx:
            emit(nc, tc, ctx)
    nc.compile()
    _CACHED_NC = nc
    return nc


def shard_inputs(x, w_q, b_q, w_k, b_k, w_v, b_v, w_o):
    f16 = lambda a: np.ascontiguousarray(a, dtype=np.float16)
    f32 = lambda a: np.ascontiguousarray(a, dtype=np.float32)
    in_maps = []
    for c in range(NCORE):
        b, g = divmod(c, 2)
        cols = slice(DPC * g, DPC * (g + 1))
        xT = np.asarray(x[b], dtype=np.float16).T  # [1024, 2048]
        wo_c = np.asarray(w_o[cols, :], np.float16)  # [512, 1024]

        def src(kind, key):
            # returns the [128, w] block whose partition dim is the k-rows
            if kind == "x":
                c, k = key
                return xT[k * 128:(k + 1) * 128, c * 512:(c + 1) * 512]
            if kind == "wv":
                return w_v[key * 128:(key + 1) * 128, cols]
            if kind == "wq":
                return w_q[key * 128:(key + 1) * 128, cols]
            if kind == "wk":
                return w_k[key * 128:(key + 1) * 128, cols]
            return wo_c[key * 128:(key + 1) * 128, :]

        allin = np.empty((128, INCOLS), dtype=np.float16)
        for kind, key, w in _LAYOUT:
            o = COL[(kind, key)]
            allin[:, o:o + w] = src(kind, key)
        bqk = np.concatenate([np.asarray(b_q[cols], np.float32).reshape(4, 128).T,
                              np.asarray(b_k[cols], np.float32).reshape(4, 128).T],
                             axis=1)
        in_maps.append({
            "allin": f16(allin),
            "bqkin": f32(bqk),
            "bv": f32(b_v[cols]),
        })
    return in_maps


def unpack_out(o):
    """[128, 16, 2, 512] fp16 device layout -> [2048, 1024] fp32."""
    o = np.asarray(o, dtype=np.float32)
    return o.transpose(1, 0, 2, 3).reshape(S, D)


def gather_output(results, b_o):
    return np.stack(
        [unpack_out(results[2 * b]["out"]) + unpack_out(results[2 * b + 1]["out"])
         + np.asarray(b_o, np.float32) for b in range(B)]
    ).astype(np.float32)


def kernel(**inputs):
    f = lambda name: np.asarray(inputs[name], dtype=np.float32)
    x, w_q, b_q, w_k, b_k, w_v, b_v, w_o, b_o = (
        f("x"), f("w_q"), f("b_q"), f("w_k"), f("b_k"),
        f("w_v"), f("b_v"), f("w_o"), f("b_o"))
    nc = build_program()
    in_maps = shard_inputs(x, w_q, b_q, w_k, b_k, w_v, b_v, w_o)
    res = bass_utils.run_bass_kernel_spmd(nc, in_maps, core_ids=list(range(NCORE)))
    return gather_output(res.results, b_o)
